# revision 28
# baseline (speedup 1.0000x reference)
"""Behler-Parrinello NN on Trainium2: 8-core data-parallel Bass/Tile kernel.

Strategy
--------
- Shard the atom axis N across 8 cores (each core: 4 types x 16384 atoms,
  128 blocks of 512 atoms).  Host pre-transposes x to [T, F, Nshard] fp16;
  the MLP runs on the PE with atoms on the moving axis; PSUM accumulates f32.
- Layer 2 (500x200, 54% of the MACs) runs in fp8-e4m3 with DoubleRow perf
  mode: K=512 contracts as 2 passes of 2x128 packed rows -> 4 matmuls
  instead of 8.  h1 is written in fp8 directly by the relu ops (scaled by
  A1=16; the DVE half is additionally center-shifted by -C to use the e4m3
  sign bit, with the shift folded into the L2 bias); W2 is host-quantized
  (x16).  All scales are powers of two and ride the positively-homogeneous
  relus (z2/h2/z3/h3 carry a 256x scale, removed by scaling w4 by 1/256),
  so no extra scaling ops exist anywhere.  Layers 1/3/4 + the scatter stay
  fp16 (end-to-end rel-err 1.84e-2 vs the 2e-2 gate, dominated by the fp8
  quantization; the fp16-only baseline is 3.9e-4).
- The scatter e[ind] += v is matmuls on one-hot matrices.  Both one-hots
  depend only on ind, so the host precomputes them (Q over q=ind//128,
  R over r=ind%128) and ships them with x in ONE fused DMA per block
  (xqr = [x | Q | R], [128, 3, 512] fp16).  On device only a single
  broadcast-multiply remains (A = Q * v, v read straight from PSUM), then
  eT_rq += R_g^T @ A_g accumulates in one persistent PSUM bank (the host
  transposes at the end).  Per-atom energies v come from per-group
  matmuls v = h3_g^T @ w4 interleaved between the L1 matmuls.
- 6-deep software pipeline, three-way engine balance (each ~2.76us/block,
  ~90% busy; per-engine queue order chosen so every op's inputs are ready
  when the engine reaches it):
    PE  : L1(p)+L4(p-4) | L2dr(p-1) | scatter(p-6) | L3(p-2)
    ACT : relu-c0(p) | L2-relu-mc0(p-1) | relu-c1(p) | h3-relu(p-2)
    DVE : relu-c2(p) | A-mult(p-4) | relu-c3(p) | L2-relu-mc1(p-1)
- PSUM banks: z1a(2) + z1b(2) + z2a(1, reused for z3) + z2b(1) + v(1)
  + e(1) = 8 (all of PSUM).
- Host sums the 8 per-core partial e grids in float64; b4 is applied on
  the host via per-molecule atom counts (exact; b4 is zero here anyway).
"""

import os
from contextlib import ExitStack

import numpy as np
import ml_dtypes

import concourse.bacc as bacc
import concourse.mybir as mybir
import concourse.tile as tile
from concourse.bass_utils import run_bass_kernel_spmd

F32 = mybir.dt.float32
F16 = mybir.dt.float16
F8 = mybir.dt.float8e4
AF = mybir.ActivationFunctionType
ALU = mybir.AluOpType
NPF8 = ml_dtypes.float8_e4m3

T, F = 4, 128
H1, H2, H3 = 500, 200, 100
MOLS = 16384
NCORES = 8
NFULL = 131072
NSHARD = NFULL // NCORES  # 16384 atoms per type per core
BLK = 512                 # atoms per block
GPB = BLK // 128          # 128-atom groups per block

H1P = 512   # padded H1 (4 kc chunks of 128)
MC = 112    # L2 output chunk (H2 200 -> 2 x 112 padded)
A1 = 16.0   # h1 fp8 scale
S2 = 16.0   # W2 fp8 scale
SCL = A1 * S2   # 256: scale carried by z2/h2/z3/h3/v
CSH = 6.0   # center shift for the DVE half of h1 (kc2/kc3)


def build_program(ns=NSHARD, t_types=T):
    assert ns % BLK == 0
    nblk = ns // BLK           # 32 blocks per type
    NB = t_types * nblk        # 128 data blocks
    X = ns // 128              # 128 groups per type

    nc = bacc.Bacc(
        "TRN2", target_bir_lowering=False, debug=False, enable_asserts=False
    )

    def din(name, shape, dt=F16):
        return nc.dram_tensor(name, shape, dt, kind="ExternalInput").ap()

    xqr = din("xqr", [t_types, 128, ns // BLK, 3, BLK])
    w1t = din("w1t", [t_types, F, H1P])
    w2t = din("w2t", [t_types, 128, 2 * 2 * 2 * MC], F8)
    w34t = din("w34t", [t_types, MC, 2 * MC + 2])
    ball = din("ball", [t_types, 128, 7], F32)
    eout = nc.dram_tensor("e_part", [128, 128], F32, kind="ExternalOutput").ap()

    n_scatter = NB * GPB
    state = dict(scnt=0)

    with tile.TileContext(nc) as tc:
        with ExitStack() as ctx:
            const = ctx.enter_context(tc.tile_pool(name="const", bufs=1))
            wpool = ctx.enter_context(tc.tile_pool(name="w", bufs=2))
            hpool = ctx.enter_context(tc.tile_pool(name="h", bufs=3))
            h3pool = ctx.enter_context(tc.tile_pool(name="h3", bufs=4))
            abpool = ctx.enter_context(tc.tile_pool(name="ab", bufs=4))
            ropool = ctx.enter_context(tc.tile_pool(name="ro", bufs=10))
            zpool = ctx.enter_context(tc.tile_pool(name="z", bufs=1, space="PSUM"))
            vpool = ctx.enter_context(tc.tile_pool(name="v", bufs=1, space="PSUM"))
            epool = ctx.enter_context(tc.tile_pool(name="e", bufs=1, space="PSUM"))

            # persistent PSUM e accumulator
            e_ps = epool.tile([128, 128], F32, tag="eacc")

            # HAM warm-up: dummy matmuls on uninitialized SBUF while the
            # first DMAs are in flight, so the PE clock gate is already at
            # 8/8 when real work starts.  Results land in the z banks and
            # are discarded by the first real start=True writes.
            warm_sb = const.tile([128, BLK], F16, tag="warm")
            nc.gpsimd.memset(warm_sb[:], 1.0)
            wa = zpool.tile([128, 2, BLK], F32, tag="z1a", name="warm_a")
            wb = zpool.tile([128, 2, BLK], F32, tag="z1b", name="warm_b")
            for i in range(2):
                nc.tensor.matmul(wa[:, i, :], lhsT=warm_sb[:, :128],
                                 rhs=warm_sb[:], start=True, stop=True)
                nc.tensor.matmul(wb[:, i, :], lhsT=warm_sb[:, :128],
                                 rhs=warm_sb[:], start=True, stop=True)
            wc = zpool.tile([MC, BLK], F32, tag="z20", name="warm_c")
            nc.tensor.matmul(wc[:], lhsT=warm_sb[:, :MC], rhs=warm_sb[:],
                             start=True, stop=True)
            wd = zpool.tile([MC, BLK], F32, tag="z21", name="warm_d")
            nc.tensor.matmul(wd[:], lhsT=warm_sb[:, :MC], rhs=warm_sb[:],
                             start=True, stop=True)

            wt = {}

            def load_type_rest(t, d):
                d["w2"] = wpool.tile([128, 2, 2, 2, MC], F8, tag="w2", name=f"w2_{t}")
                nc.sync.dma_start(
                    d["w2"][:],
                    w2t[t].rearrange("p (a b c m) -> p a b c m", a=2, b=2, c=2),
                )
                # w3 [112, 2, 112] fp16 with w4 packed in the tail columns
                w34 = wpool.tile([MC, 2 * MC + 2], F16, tag="w34", name=f"w34_{t}")
                nc.sync.dma_start(w34[:], w34t[t])
                d["w3"] = w34[:, :2 * MC].rearrange("p (k m) -> p k m", k=2)
                d["w4"] = w34[:H3, 2 * MC:2 * MC + 1]
                # all biases in one [128, 7] f32 tile:
                # cols 0,1 = b1a; 2,3 = b1d; 4,5 = b2 (112 rows); 6 = b3
                bb = wpool.tile([128, 7], F32, tag="bb", name=f"bb_{t}")
                nc.sync.dma_start(bb[:], ball[t])
                d["b1a"] = bb[:, 0:2]
                d["b1d"] = bb[:, 2:4]
                d["b2"] = bb[:MC, 4:6]
                d["b3"] = bb[:MC, 6:7]

            # per-stage persistent handles, keyed by data-block index
            S = {}

            def stage_l1(j):
                t, b = j // nblk, j % nblk
                fresh = t not in wt
                if fresh:
                    # w1 first so the very first L1 matmul isn't stuck
                    # behind the whole weight-DMA train
                    d0 = {}
                    d0["w1"] = wpool.tile([F, H1P], F16, tag="w1",
                                          name=f"w1_{t}")
                    nc.sync.dma_start(d0["w1"][:], w1t[t])
                    wt[t] = d0
                d = wt[t]
                xq = ropool.tile([128, 3, BLK], F16, tag="xqr", name=f"xq{j}")
                nc.sync.dma_start(xq[:], xqr[t, :, b])
                xt = xq[:, 0, :]
                qo = xq[:, 1, :].rearrange("p (g s) -> p g s", g=GPB)
                ro = xq[:, 2, :].rearrange("p (g s) -> p g s", g=GPB)
                if fresh:
                    load_type_rest(t, d)
                z1a = zpool.tile([128, 2, BLK], F32, tag="z1a", name=f"z1a{j}")
                z1b = zpool.tile([128, 2, BLK], F32, tag="z1b", name=f"z1b{j}")
                h1 = hpool.tile([128, 4, BLK], F8, tag="h1", name=f"h1{j}")
                S[j] = dict(t=t, b=b, z1a=z1a, z1b=z1b, h1=h1, qo=qo, ro=ro,
                            xt=xt)

            def stage_l1_mm(j, c):
                st = S[j]
                d = wt[st["t"]]
                dst = st["z1a"] if c < 2 else st["z1b"]
                nc.tensor.matmul(
                    dst[:, c % 2, :],
                    lhsT=d["w1"][:, c * 128:(c + 1) * 128],
                    rhs=st["xt"],
                    start=True,
                    stop=True,
                )

            def stage_relu_act(j, c):
                # relu chunk c (0/1) on ACT (unshifted)
                st = S[j]
                d = wt[st["t"]]
                nc.scalar.activation(
                    st["h1"][:, c, :], st["z1a"][:, c, :],
                    AF.Relu, bias=d["b1a"][:, c:c + 1],
                )

            def stage_relu_dve(j, c):
                # relu chunk 2+c on DVE, center-shifted: max(z+a1*b1-C, -C)
                st = S[j]
                d = wt[st["t"]]
                nc.vector.tensor_scalar(
                    st["h1"][:, 2 + c, :], st["z1b"][:, c, :],
                    d["b1d"][:, c:c + 1], -CSH,
                    op0=ALU.add, op1=ALU.max,
                )
                if c == 1:
                    del st["z1a"], st["z1b"]

            def stage_l2(j):
                st = S[j]
                d = wt[st["t"]]
                z2 = [
                    zpool.tile([MC, BLK], F32, tag=f"z2{mc}", name=f"z2{mc}_{j}")
                    for mc in range(2)
                ]
                for mc in range(2):
                    for kc2 in range(2):
                        nc.tensor.matmul(
                            z2[mc][:],
                            lhsT=d["w2"][:, kc2, mc],
                            rhs=st["h1"][:, 2 * kc2:2 * kc2 + 2, :],
                            start=(kc2 == 0),
                            stop=(kc2 == 1),
                            perf_mode=mybir.MatmulPerfMode.DoubleRow,
                        )
                st["z2"] = z2
                st["h2"] = hpool.tile([MC, 2, BLK], F16, tag="h2", name=f"h2{j}")

            def stage_relu_l2_act(j):
                st = S[j]
                d = wt[st["t"]]
                nc.scalar.activation(
                    st["h2"][:, 0, :], st["z2"][0][:],
                    AF.Relu, bias=d["b2"][:, 0:1],
                )

            def stage_relu_l2_dve(j):
                st = S[j]
                d = wt[st["t"]]
                nc.vector.tensor_scalar(
                    st["h2"][:, 1, :], st["z2"][1][:],
                    d["b2"][:, 1:2], 0.0,
                    op0=ALU.add, op1=ALU.max,
                )
                del st["z2"]

            def stage_l3(j):
                st = S[j]
                d = wt[st["t"]]
                # z3 reuses the z2a bank (its reader l2r-mc0 runs at the
                # start of the period, a full period before these matmuls)
                z3 = zpool.tile([MC, BLK], F32, tag="z20", name=f"z3{j}")
                for kc in range(2):
                    nc.tensor.matmul(
                        z3[:],
                        lhsT=d["w3"][:, kc, :],
                        rhs=st["h2"][:, kc, :],
                        start=(kc == 0),
                        stop=(kc == 1),
                    )
                st["z3"] = z3

            def stage_relu_l3(j):
                st = S[j]
                d = wt[st["t"]]
                h3 = h3pool.tile([H3, BLK], F16, tag="h3", name=f"h3{j}")
                nc.scalar.activation(
                    h3[:], st["z3"][:H3, :], AF.Relu, bias=d["b3"][:H3, :]
                )
                st["h3"] = h3
                del st["z3"]

            def stage_l4(j, gs):
                st = S[j]
                d = wt[st["t"]]
                if gs == 0:
                    st["v_ps"] = vpool.tile([128, GPB], F32, tag="v",
                                            name=f"v{j}")
                for g in (gs, gs + 1):
                    nc.tensor.matmul(
                        st["v_ps"][:, g:g + 1],
                        lhsT=st["h3"][:, g * 128:(g + 1) * 128],
                        rhs=d["w4"],
                        start=(g == 0),
                        stop=(g == GPB - 1),
                    )

            def stage_builds(j):
                st = S[j]
                a_sb = abpool.tile([128, GPB, 128], F16, tag="A", name=f"a{j}")
                vb = (st["v_ps"][:].unsqueeze(2)
                      .broadcast_to([128, GPB, 128]))
                nc.vector.tensor_tensor(a_sb[:], st["qo"], vb, op=ALU.mult)
                st["A"] = a_sb
                del st["v_ps"], st["h3"], st["qo"]

            def stage_scatter(j):
                st = S[j]
                for g in range(GPB):
                    nc.tensor.matmul(
                        e_ps[:],
                        lhsT=st["ro"][:, g],
                        rhs=st["A"][:, g],
                        start=(state["scnt"] == 0),
                        stop=(state["scnt"] == n_scatter - 1),
                    )
                    state["scnt"] += 1
                del S[j]

            # ---- software pipeline ----
            # PE : L1(p)+L4(p-4) | L2(p-1) | scatter(p-6) | L3(p-2)
            # ACT: c0(p) | l2r-mc0(p-1) | c1(p) | h3r(p-2)
            # DVE: c2(p) | A-mult(p-4) | c3(p) | l2r-mc1(p-1)
            def valid(i):
                return 0 <= i < NB

            for p in range(NB + 7):
                if valid(p - 6):
                    stage_scatter(p - 6)
                if valid(p):
                    stage_l1(p)
                    stage_l1_mm(p, 2)
                if valid(p - 4):
                    stage_l4(p - 4, 0)
                if valid(p):
                    stage_l1_mm(p, 3)
                if valid(p - 4):
                    stage_l4(p - 4, 2)
                if valid(p):
                    stage_l1_mm(p, 0)
                    stage_l1_mm(p, 1)
                    stage_relu_act(p, 0)
                if valid(p - 1):
                    stage_l2(p - 1)
                    stage_relu_l2_act(p - 1)
                if valid(p):
                    stage_relu_act(p, 1)
                    stage_relu_dve(p, 0)
                if valid(p - 4):
                    stage_builds(p - 4)
                if valid(p):
                    stage_relu_dve(p, 1)
                if valid(p - 1):
                    stage_relu_l2_dve(p - 1)
                if valid(p - 2):
                    stage_l3(p - 2)
                    stage_relu_l3(p - 2)

            e_sb = const.tile([128, 128], F32, tag="eout")
            nc.vector.tensor_copy(e_sb[:], e_ps[:])
            nc.sync.dma_start(eout, e_sb[:])

    nc.compile()
    return nc


def q8(x, clip=240.0):
    return np.clip(x, -clip, clip).astype(NPF8)


def prep_shared(W1, b1, W2, b2, W3, b3, W4, b4):
    """Weight/bias layout marshaling (replicated across cores)."""
    f = np.float32
    h = np.float16

    # L1: W1 scaled by A1, padded 500->512, transposed to [T, F, H1P]
    w1p = np.zeros((T, H1P, F), dtype=f)
    w1p[:, :H1, :] = A1 * W1
    w1t = np.ascontiguousarray(w1p.transpose(0, 2, 1), dtype=h)

    # L2: q8(S2*W2) padded to [T, 224, 512], laid out for DoubleRow:
    # [t, p, kc2, mc, i, m] = w2q[t, mc*112+m, (kc2*2+i)*128 + p]
    w2p = np.zeros((T, 2 * MC, H1P), dtype=f)
    w2p[:, :H2, :H1] = S2 * W2
    w2q = q8(w2p)
    w2qf = w2q.astype(f)
    w2r = (
        w2q.reshape(T, 2, MC, 2, 2, 128)     # [t, mc, m, kc2, i, p]
        .transpose(0, 5, 3, 1, 4, 2)         # [t, p, kc2, mc, i, m]
        .reshape(T, 128, 2 * 2 * 2 * MC)
    )
    w2t = np.ascontiguousarray(w2r)

    # L3: W3 padded [T, 112(m), 224(k)] -> [t, p, kc, m] = W3p[m, kc*112+p]
    w3p = np.zeros((T, MC, 2 * MC), dtype=f)
    w3p[:, :H3, :H2] = W3
    w3r = w3p.reshape(T, MC, 2, MC).transpose(0, 3, 2, 1).reshape(T, MC, 2 * MC)
    w34t = np.zeros((T, MC, 2 * MC + 2), dtype=h)
    w34t[:, :, :2 * MC] = w3r.astype(h)
    w34t[:, :H3, 2 * MC] = (W4 / SCL).reshape(T, H3).astype(h)
    w34t = np.ascontiguousarray(w34t)

    b1p = np.zeros((T, H1P), dtype=f)
    b1p[:, :H1] = A1 * b1
    b1c = b1p.reshape(T, 4, 128).transpose(0, 2, 1)  # [T, 128, 4]

    # b2 folded: 256*b2 + C * sum_{k in shifted half} w2q[m, k]
    b2p = np.zeros((T, 2 * MC), dtype=f)
    b2p[:, :H2] = SCL * b2
    b2p += CSH * w2qf[:, :, 256:512].sum(axis=2)  # shift correction [T, 224]
    b2c = b2p.reshape(T, 2, MC).transpose(0, 2, 1)  # [T, 112, 2]

    b3p = np.zeros((T, MC), dtype=f)
    b3p[:, :H3] = SCL * b3

    ball = np.zeros((T, 128, 7), dtype=f)
    ball[:, :, 0:2] = b1c[:, :, 0:2]
    ball[:, :, 2:4] = b1c[:, :, 2:4] - CSH
    ball[:, :MC, 4:6] = b2c
    ball[:, :MC, 6] = b3p

    out = {
        "w1t": w1t,
        "w2t": w2t,
        "w34t": w34t,
        "ball": ball,
    }
    return out


def prep_core(x, ind, core, ns=NSHARD):
    """Per-core shard marshaling: transposed x and split/transposed indices."""
    h = np.float16
    sl = slice(core * ns, (core + 1) * ns)
    X = ns // 128
    NBK = ns // 512
    xs = x[:, sl, :]
    xT = xs.transpose(0, 2, 1).astype(h)               # [t, p, n]
    inds = np.asarray(ind[:, sl]).astype(np.int64)
    q = (inds // 128).reshape(T, X, 128)               # [t, x, p]
    r = (inds % 128).reshape(T, X, 128)
    qoh = np.zeros((T, X, 128, 128), dtype=h)          # [t, x, p, s]
    roh = np.zeros((T, X, 128, 128), dtype=h)
    np.put_along_axis(qoh, q[..., None], 1.0, axis=3)
    np.put_along_axis(roh, r[..., None], 1.0, axis=3)
    qoh = qoh.transpose(0, 2, 1, 3).reshape(T, 128, ns)  # [t, p, x*128+s]
    roh = roh.transpose(0, 2, 1, 3).reshape(T, 128, ns)
    xqr = np.empty((T, 128, NBK, 3, 512), dtype=h)
    xqr[:, :, :, 0, :] = xT.reshape(T, 128, NBK, 512)
    xqr[:, :, :, 1, :] = qoh.reshape(T, 128, NBK, 512)
    xqr[:, :, :, 2, :] = roh.reshape(T, 128, NBK, 512)
    return {"xqr": np.ascontiguousarray(xqr)}


_CACHE = {}


def _get_program():
    if "nc" not in _CACHE:
        _CACHE["nc"] = build_program()
    return _CACHE["nc"]


def _ensure_ntff_hook():
    """Install the axon NTFF profile hook if the image's antenv lacks it."""
    import sys
    import types

    try:
        from antenv.axon_hooks import get_axon_ntff_profile_hook  # noqa: F401
        return
    except ImportError:
        pass
    try:
        from trn_agent_boot.trn_boot import _ntff_profile_via_ctypes
    except ImportError:
        return
    so = "/opt/axon/libaxon_pjrt.so"
    if not os.path.exists(so):
        return
    hook = _ntff_profile_via_ctypes(so)
    mod = types.ModuleType("antenv.axon_hooks")
    state = {"hook": hook}
    mod.get_axon_ntff_profile_hook = lambda: state["hook"]
    mod.set_axon_ntff_profile_hook = lambda h: state.update(hook=h)
    sys.modules["antenv.axon_hooks"] = mod


def run(inputs, trace=False, trace_kwargs=None):
    """Run the 8-core kernel. Returns (out [M,1] f32, BassKernelResults)."""
    x = np.asarray(inputs["x"], dtype=np.float32)
    ind = np.asarray(inputs["ind"])
    e = np.asarray(inputs["e"], dtype=np.float32)
    b4 = np.asarray(inputs["b4"], dtype=np.float64)
    shared = prep_shared(
        np.asarray(inputs["W1"]), np.asarray(inputs["b1"]),
        np.asarray(inputs["W2"]), np.asarray(inputs["b2"]),
        np.asarray(inputs["W3"]), np.asarray(inputs["b3"]),
        np.asarray(inputs["W4"]), np.asarray(inputs["b4"]),
    )
    in_maps = []
    for c in range(NCORES):
        m = dict(shared)
        m.update(prep_core(x, ind, c))
        in_maps.append(m)

    nc = _get_program()
    if trace:
        _ensure_ntff_hook()
    res = run_bass_kernel_spmd(
        nc,
        in_maps,
        core_ids=list(range(NCORES)),
        trace=trace,
        **(trace_kwargs or {}),
    )
    acc = e.reshape(-1).astype(np.float64).copy()
    for rm in res.results:
        acc += rm["e_part"].astype(np.float64).T.reshape(-1)
    # b4 applied host-side: each atom of type t contributes +b4[t]
    if np.any(b4 != 0.0):
        for t in range(T):
            acc += np.bincount(
                np.asarray(ind[t]).reshape(-1), minlength=MOLS
            ) * float(b4[t])
    out = acc.astype(np.float32).reshape(MOLS, 1)
    return out, res


def kernel(**inputs):
    out, _ = run(inputs, trace=False)
    return out


# revision 29
# speedup vs baseline: 1.0068x; 1.0068x over previous
"""Behler-Parrinello NN on Trainium2: 8-core data-parallel Bass/Tile kernel.

Strategy
--------
- Shard the atom axis N across 8 cores (each core: 4 types x 16384 atoms,
  128 blocks of 512 atoms).  Host pre-transposes x to [T, F, Nshard] fp16;
  the MLP runs on the PE with atoms on the moving axis; PSUM accumulates f32.
- Layer 2 (500x200, 54% of the MACs) runs in fp8-e4m3 with DoubleRow perf
  mode: K=512 contracts as 2 passes of 2x128 packed rows -> 4 matmuls
  instead of 8.  h1 is written in fp8 directly by the relu ops (scaled by
  A1=16; the DVE half is additionally center-shifted by -C to use the e4m3
  sign bit, with the shift folded into the L2 bias); W2 is host-quantized
  (x16).  All scales are powers of two and ride the positively-homogeneous
  relus (z2/h2/z3/h3 carry a 256x scale, removed by scaling w4 by 1/256),
  so no extra scaling ops exist anywhere.  Layers 1/3/4 + the scatter stay
  fp16 (end-to-end rel-err 1.84e-2 vs the 2e-2 gate, dominated by the fp8
  quantization; the fp16-only baseline is 3.9e-4).
- The scatter e[ind] += v is matmuls on one-hot matrices.  Both one-hots
  depend only on ind, so the host precomputes them (Q over q=ind//128,
  R over r=ind%128) and ships them with x in ONE fused DMA per block
  (xqr = [x | Q | R], [128, 3, 512] fp16).  On device only a single
  broadcast-multiply remains (A = Q * v, v read straight from PSUM), then
  eT_rq += R_g^T @ A_g accumulates in one persistent PSUM bank (the host
  transposes at the end).  Per-atom energies v come from per-group
  matmuls v = h3_g^T @ w4 interleaved between the L1 matmuls.
- 6-deep software pipeline, three-way engine balance (each ~2.76us/block,
  ~90% busy; per-engine queue order chosen so every op's inputs are ready
  when the engine reaches it):
    PE  : L1(p)+L4(p-4) | L2dr(p-1) | scatter(p-6) | L3(p-2)
    ACT : relu-c0(p) | L2-relu-mc0(p-1) | relu-c1(p) | h3-relu(p-2)
    DVE : relu-c2(p) | A-mult(p-4) | relu-c3(p) | L2-relu-mc1(p-1)
- PSUM banks: z1a(2) + z1b(2) + z2a(1, reused for z3) + z2b(1) + v(1)
  + e(1) = 8 (all of PSUM).
- Host sums the 8 per-core partial e grids in float64; b4 is applied on
  the host via per-molecule atom counts (exact; b4 is zero here anyway).
"""

import os
from contextlib import ExitStack

import numpy as np
import ml_dtypes

import concourse.bacc as bacc
import concourse.mybir as mybir
import concourse.tile as tile
from concourse.bass_utils import run_bass_kernel_spmd

F32 = mybir.dt.float32
F16 = mybir.dt.float16
F8 = mybir.dt.float8e4
AF = mybir.ActivationFunctionType
ALU = mybir.AluOpType
NPF8 = ml_dtypes.float8_e4m3

T, F = 4, 128
H1, H2, H3 = 500, 200, 100
MOLS = 16384
NCORES = 8
NFULL = 131072
NSHARD = NFULL // NCORES  # 16384 atoms per type per core
BLK = 512                 # atoms per block
GPB = BLK // 128          # 128-atom groups per block

H1P = 512   # padded H1 (4 kc chunks of 128)
MC = 112    # L2 output chunk (H2 200 -> 2 x 112 padded)
A1 = 16.0   # h1 fp8 scale
S2 = 16.0   # W2 fp8 scale
SCL = A1 * S2   # 256: scale carried by z2/h2/z3/h3/v
CSH = 6.0   # center shift for the DVE half of h1 (kc2/kc3)


def build_program(ns=NSHARD, t_types=T):
    assert ns % BLK == 0
    nblk = ns // BLK           # 32 blocks per type
    NB = t_types * nblk        # 128 data blocks
    X = ns // 128              # 128 groups per type

    nc = bacc.Bacc(
        "TRN2", target_bir_lowering=False, debug=False, enable_asserts=False
    )

    def din(name, shape, dt=F16):
        return nc.dram_tensor(name, shape, dt, kind="ExternalInput").ap()

    xqr = din("xqr", [t_types, 128, ns // BLK, 3, BLK])
    w1t = din("w1t", [t_types, F, H1P])
    w2t = din("w2t", [t_types, 128, 2 * 2 * 2 * MC], F8)
    w34t = din("w34t", [t_types, MC, 2 * MC + 2])
    ball = din("ball", [t_types, 128, 7], F32)
    eout = nc.dram_tensor("e_part", [128, 128], F32, kind="ExternalOutput").ap()

    n_scatter = NB * GPB
    state = dict(scnt=0)

    with tile.TileContext(nc) as tc:
        with ExitStack() as ctx:
            const = ctx.enter_context(tc.tile_pool(name="const", bufs=1))
            wpool = ctx.enter_context(tc.tile_pool(name="w", bufs=2))
            hpool = ctx.enter_context(tc.tile_pool(name="h", bufs=3))
            h3pool = ctx.enter_context(tc.tile_pool(name="h3", bufs=4))
            abpool = ctx.enter_context(tc.tile_pool(name="ab", bufs=4))
            ropool = ctx.enter_context(tc.tile_pool(name="ro", bufs=10))
            zpool = ctx.enter_context(tc.tile_pool(name="z", bufs=1, space="PSUM"))
            vpool = ctx.enter_context(tc.tile_pool(name="v", bufs=1, space="PSUM"))
            epool = ctx.enter_context(tc.tile_pool(name="e", bufs=1, space="PSUM"))

            # persistent PSUM e accumulator
            e_ps = epool.tile([128, 128], F32, tag="eacc")

            wt = {}

            def load_type_rest(t, d):
                d["w2"] = wpool.tile([128, 2, 2, 2, MC], F8, tag="w2", name=f"w2_{t}")
                nc.sync.dma_start(
                    d["w2"][:],
                    w2t[t].rearrange("p (a b c m) -> p a b c m", a=2, b=2, c=2),
                )
                # w3 [112, 2, 112] fp16 with w4 packed in the tail columns
                w34 = wpool.tile([MC, 2 * MC + 2], F16, tag="w34", name=f"w34_{t}")
                nc.sync.dma_start(w34[:], w34t[t])
                d["w3"] = w34[:, :2 * MC].rearrange("p (k m) -> p k m", k=2)
                d["w4"] = w34[:H3, 2 * MC:2 * MC + 1]
                # all biases in one [128, 7] f32 tile:
                # cols 0,1 = b1a; 2,3 = b1d; 4,5 = b2 (112 rows); 6 = b3
                bb = wpool.tile([128, 7], F32, tag="bb", name=f"bb_{t}")
                nc.sync.dma_start(bb[:], ball[t])
                d["b1a"] = bb[:, 0:2]
                d["b1d"] = bb[:, 2:4]
                d["b2"] = bb[:MC, 4:6]
                d["b3"] = bb[:MC, 6:7]

            # per-stage persistent handles, keyed by data-block index
            S = {}

            def stage_l1(j):
                t, b = j // nblk, j % nblk
                fresh = t not in wt
                if fresh:
                    # w1 first so the very first L1 matmul isn't stuck
                    # behind the whole weight-DMA train
                    d0 = {}
                    d0["w1"] = wpool.tile([F, H1P], F16, tag="w1",
                                          name=f"w1_{t}")
                    nc.sync.dma_start(d0["w1"][:], w1t[t])
                    wt[t] = d0
                d = wt[t]
                xq = ropool.tile([128, 3, BLK], F16, tag="xqr", name=f"xq{j}")
                nc.sync.dma_start(xq[:], xqr[t, :, b])
                xt = xq[:, 0, :]
                qo = xq[:, 1, :].rearrange("p (g s) -> p g s", g=GPB)
                ro = xq[:, 2, :].rearrange("p (g s) -> p g s", g=GPB)
                if fresh:
                    load_type_rest(t, d)
                z1a = zpool.tile([128, 2, BLK], F32, tag="z1a", name=f"z1a{j}")
                z1b = zpool.tile([128, 2, BLK], F32, tag="z1b", name=f"z1b{j}")
                h1 = hpool.tile([128, 4, BLK], F8, tag="h1", name=f"h1{j}")
                S[j] = dict(t=t, b=b, z1a=z1a, z1b=z1b, h1=h1, qo=qo, ro=ro,
                            xt=xt)

            def stage_l1_mm(j, c):
                st = S[j]
                d = wt[st["t"]]
                dst = st["z1a"] if c < 2 else st["z1b"]
                nc.tensor.matmul(
                    dst[:, c % 2, :],
                    lhsT=d["w1"][:, c * 128:(c + 1) * 128],
                    rhs=st["xt"],
                    start=True,
                    stop=True,
                )

            def stage_relu_act(j, c):
                # relu chunk c (0/1) on ACT (unshifted)
                st = S[j]
                d = wt[st["t"]]
                nc.scalar.activation(
                    st["h1"][:, c, :], st["z1a"][:, c, :],
                    AF.Relu, bias=d["b1a"][:, c:c + 1],
                )

            def stage_relu_dve(j, c):
                # relu chunk 2+c on DVE, center-shifted: max(z+a1*b1-C, -C)
                st = S[j]
                d = wt[st["t"]]
                nc.vector.tensor_scalar(
                    st["h1"][:, 2 + c, :], st["z1b"][:, c, :],
                    d["b1d"][:, c:c + 1], -CSH,
                    op0=ALU.add, op1=ALU.max,
                )
                if c == 1:
                    del st["z1a"], st["z1b"]

            def stage_l2(j):
                st = S[j]
                d = wt[st["t"]]
                z2 = [
                    zpool.tile([MC, BLK], F32, tag=f"z2{mc}", name=f"z2{mc}_{j}")
                    for mc in range(2)
                ]
                for mc in range(2):
                    for kc2 in range(2):
                        nc.tensor.matmul(
                            z2[mc][:],
                            lhsT=d["w2"][:, kc2, mc],
                            rhs=st["h1"][:, 2 * kc2:2 * kc2 + 2, :],
                            start=(kc2 == 0),
                            stop=(kc2 == 1),
                            perf_mode=mybir.MatmulPerfMode.DoubleRow,
                        )
                st["z2"] = z2
                st["h2"] = hpool.tile([MC, 2, BLK], F16, tag="h2", name=f"h2{j}")

            def stage_relu_l2_act(j):
                st = S[j]
                d = wt[st["t"]]
                nc.scalar.activation(
                    st["h2"][:, 0, :], st["z2"][0][:],
                    AF.Relu, bias=d["b2"][:, 0:1],
                )

            def stage_relu_l2_dve(j):
                st = S[j]
                d = wt[st["t"]]
                nc.vector.tensor_scalar(
                    st["h2"][:, 1, :], st["z2"][1][:],
                    d["b2"][:, 1:2], 0.0,
                    op0=ALU.add, op1=ALU.max,
                )
                del st["z2"]

            def stage_l3(j):
                st = S[j]
                d = wt[st["t"]]
                # z3 reuses the z2a bank (its reader l2r-mc0 runs at the
                # start of the period, a full period before these matmuls)
                z3 = zpool.tile([MC, BLK], F32, tag="z20", name=f"z3{j}")
                for kc in range(2):
                    nc.tensor.matmul(
                        z3[:],
                        lhsT=d["w3"][:, kc, :],
                        rhs=st["h2"][:, kc, :],
                        start=(kc == 0),
                        stop=(kc == 1),
                    )
                st["z3"] = z3

            def stage_relu_l3(j):
                st = S[j]
                d = wt[st["t"]]
                h3 = h3pool.tile([H3, BLK], F16, tag="h3", name=f"h3{j}")
                nc.scalar.activation(
                    h3[:], st["z3"][:H3, :], AF.Relu, bias=d["b3"][:H3, :]
                )
                st["h3"] = h3
                del st["z3"]

            def stage_l4(j, gs):
                st = S[j]
                d = wt[st["t"]]
                if gs == 0:
                    st["v_ps"] = vpool.tile([128, GPB], F32, tag="v",
                                            name=f"v{j}")
                for g in (gs, gs + 1):
                    nc.tensor.matmul(
                        st["v_ps"][:, g:g + 1],
                        lhsT=st["h3"][:, g * 128:(g + 1) * 128],
                        rhs=d["w4"],
                        start=(g == 0),
                        stop=(g == GPB - 1),
                    )

            def stage_builds(j):
                st = S[j]
                a_sb = abpool.tile([128, GPB, 128], F16, tag="A", name=f"a{j}")
                vb = (st["v_ps"][:].unsqueeze(2)
                      .broadcast_to([128, GPB, 128]))
                nc.vector.tensor_tensor(a_sb[:], st["qo"], vb, op=ALU.mult)
                st["A"] = a_sb
                del st["v_ps"], st["h3"], st["qo"]

            def stage_scatter(j):
                st = S[j]
                for g in range(GPB):
                    nc.tensor.matmul(
                        e_ps[:],
                        lhsT=st["ro"][:, g],
                        rhs=st["A"][:, g],
                        start=(state["scnt"] == 0),
                        stop=(state["scnt"] == n_scatter - 1),
                    )
                    state["scnt"] += 1
                del S[j]

            # ---- software pipeline ----
            # PE : L1(p)+L4(p-4) | L2(p-1) | scatter(p-6) | L3(p-2)
            # ACT: c0(p) | l2r-mc0(p-1) | c1(p) | h3r(p-2)
            # DVE: c2(p) | A-mult(p-4) | c3(p) | l2r-mc1(p-1)
            def valid(i):
                return 0 <= i < NB

            for p in range(NB + 7):
                if valid(p - 6):
                    stage_scatter(p - 6)
                if valid(p):
                    stage_l1(p)
                    stage_l1_mm(p, 2)
                if valid(p - 4):
                    stage_l4(p - 4, 0)
                if valid(p):
                    stage_l1_mm(p, 3)
                if valid(p - 4):
                    stage_l4(p - 4, 2)
                if valid(p):
                    stage_l1_mm(p, 0)
                    stage_l1_mm(p, 1)
                    stage_relu_act(p, 0)
                if valid(p - 1):
                    stage_l2(p - 1)
                    stage_relu_l2_act(p - 1)
                if valid(p):
                    stage_relu_act(p, 1)
                    stage_relu_dve(p, 0)
                if valid(p - 4):
                    stage_builds(p - 4)
                if valid(p):
                    stage_relu_dve(p, 1)
                if valid(p - 1):
                    stage_relu_l2_dve(p - 1)
                if valid(p - 2):
                    stage_l3(p - 2)
                    stage_relu_l3(p - 2)

            e_sb = const.tile([128, 128], F32, tag="eout")
            nc.vector.tensor_copy(e_sb[:], e_ps[:])
            nc.sync.dma_start(eout, e_sb[:])

    nc.compile()
    return nc


def q8(x, clip=240.0):
    return np.clip(x, -clip, clip).astype(NPF8)


def prep_shared(W1, b1, W2, b2, W3, b3, W4, b4):
    """Weight/bias layout marshaling (replicated across cores)."""
    f = np.float32
    h = np.float16

    # L1: W1 scaled by A1, padded 500->512, transposed to [T, F, H1P]
    w1p = np.zeros((T, H1P, F), dtype=f)
    w1p[:, :H1, :] = A1 * W1
    w1t = np.ascontiguousarray(w1p.transpose(0, 2, 1), dtype=h)

    # L2: q8(S2*W2) padded to [T, 224, 512], laid out for DoubleRow:
    # [t, p, kc2, mc, i, m] = w2q[t, mc*112+m, (kc2*2+i)*128 + p]
    w2p = np.zeros((T, 2 * MC, H1P), dtype=f)
    w2p[:, :H2, :H1] = S2 * W2
    w2q = q8(w2p)
    w2qf = w2q.astype(f)
    w2r = (
        w2q.reshape(T, 2, MC, 2, 2, 128)     # [t, mc, m, kc2, i, p]
        .transpose(0, 5, 3, 1, 4, 2)         # [t, p, kc2, mc, i, m]
        .reshape(T, 128, 2 * 2 * 2 * MC)
    )
    w2t = np.ascontiguousarray(w2r)

    # L3: W3 padded [T, 112(m), 224(k)] -> [t, p, kc, m] = W3p[m, kc*112+p]
    w3p = np.zeros((T, MC, 2 * MC), dtype=f)
    w3p[:, :H3, :H2] = W3
    w3r = w3p.reshape(T, MC, 2, MC).transpose(0, 3, 2, 1).reshape(T, MC, 2 * MC)
    w34t = np.zeros((T, MC, 2 * MC + 2), dtype=h)
    w34t[:, :, :2 * MC] = w3r.astype(h)
    w34t[:, :H3, 2 * MC] = (W4 / SCL).reshape(T, H3).astype(h)
    w34t = np.ascontiguousarray(w34t)

    b1p = np.zeros((T, H1P), dtype=f)
    b1p[:, :H1] = A1 * b1
    b1c = b1p.reshape(T, 4, 128).transpose(0, 2, 1)  # [T, 128, 4]

    # b2 folded: 256*b2 + C * sum_{k in shifted half} w2q[m, k]
    b2p = np.zeros((T, 2 * MC), dtype=f)
    b2p[:, :H2] = SCL * b2
    b2p += CSH * w2qf[:, :, 256:512].sum(axis=2)  # shift correction [T, 224]
    b2c = b2p.reshape(T, 2, MC).transpose(0, 2, 1)  # [T, 112, 2]

    b3p = np.zeros((T, MC), dtype=f)
    b3p[:, :H3] = SCL * b3

    ball = np.zeros((T, 128, 7), dtype=f)
    ball[:, :, 0:2] = b1c[:, :, 0:2]
    ball[:, :, 2:4] = b1c[:, :, 2:4] - CSH
    ball[:, :MC, 4:6] = b2c
    ball[:, :MC, 6] = b3p

    out = {
        "w1t": w1t,
        "w2t": w2t,
        "w34t": w34t,
        "ball": ball,
    }
    return out


def prep_core(x, ind, core, ns=NSHARD):
    """Per-core shard marshaling: transposed x and split/transposed indices."""
    h = np.float16
    sl = slice(core * ns, (core + 1) * ns)
    X = ns // 128
    NBK = ns // 512
    xs = x[:, sl, :]
    xT = xs.transpose(0, 2, 1).astype(h)               # [t, p, n]
    inds = np.asarray(ind[:, sl]).astype(np.int64)
    q = (inds // 128).reshape(T, X, 128)               # [t, x, p]
    r = (inds % 128).reshape(T, X, 128)
    qoh = np.zeros((T, X, 128, 128), dtype=h)          # [t, x, p, s]
    roh = np.zeros((T, X, 128, 128), dtype=h)
    np.put_along_axis(qoh, q[..., None], 1.0, axis=3)
    np.put_along_axis(roh, r[..., None], 1.0, axis=3)
    qoh = qoh.transpose(0, 2, 1, 3).reshape(T, 128, ns)  # [t, p, x*128+s]
    roh = roh.transpose(0, 2, 1, 3).reshape(T, 128, ns)
    xqr = np.empty((T, 128, NBK, 3, 512), dtype=h)
    xqr[:, :, :, 0, :] = xT.reshape(T, 128, NBK, 512)
    xqr[:, :, :, 1, :] = qoh.reshape(T, 128, NBK, 512)
    xqr[:, :, :, 2, :] = roh.reshape(T, 128, NBK, 512)
    return {"xqr": np.ascontiguousarray(xqr)}


_CACHE = {}


def _get_program():
    if "nc" not in _CACHE:
        _CACHE["nc"] = build_program()
    return _CACHE["nc"]


def _ensure_ntff_hook():
    """Install the axon NTFF profile hook if the image's antenv lacks it."""
    import sys
    import types

    try:
        from antenv.axon_hooks import get_axon_ntff_profile_hook  # noqa: F401
        return
    except ImportError:
        pass
    try:
        from trn_agent_boot.trn_boot import _ntff_profile_via_ctypes
    except ImportError:
        return
    so = "/opt/axon/libaxon_pjrt.so"
    if not os.path.exists(so):
        return
    hook = _ntff_profile_via_ctypes(so)
    mod = types.ModuleType("antenv.axon_hooks")
    state = {"hook": hook}
    mod.get_axon_ntff_profile_hook = lambda: state["hook"]
    mod.set_axon_ntff_profile_hook = lambda h: state.update(hook=h)
    sys.modules["antenv.axon_hooks"] = mod


def run(inputs, trace=False, trace_kwargs=None):
    """Run the 8-core kernel. Returns (out [M,1] f32, BassKernelResults)."""
    x = np.asarray(inputs["x"], dtype=np.float32)
    ind = np.asarray(inputs["ind"])
    e = np.asarray(inputs["e"], dtype=np.float32)
    b4 = np.asarray(inputs["b4"], dtype=np.float64)
    shared = prep_shared(
        np.asarray(inputs["W1"]), np.asarray(inputs["b1"]),
        np.asarray(inputs["W2"]), np.asarray(inputs["b2"]),
        np.asarray(inputs["W3"]), np.asarray(inputs["b3"]),
        np.asarray(inputs["W4"]), np.asarray(inputs["b4"]),
    )
    in_maps = []
    for c in range(NCORES):
        m = dict(shared)
        m.update(prep_core(x, ind, c))
        in_maps.append(m)

    nc = _get_program()
    if trace:
        _ensure_ntff_hook()
    res = run_bass_kernel_spmd(
        nc,
        in_maps,
        core_ids=list(range(NCORES)),
        trace=trace,
        **(trace_kwargs or {}),
    )
    acc = e.reshape(-1).astype(np.float64).copy()
    for rm in res.results:
        acc += rm["e_part"].astype(np.float64).T.reshape(-1)
    # b4 applied host-side: each atom of type t contributes +b4[t]
    if np.any(b4 != 0.0):
        for t in range(T):
            acc += np.bincount(
                np.asarray(ind[t]).reshape(-1), minlength=MOLS
            ) * float(b4[t])
    out = acc.astype(np.float32).reshape(MOLS, 1)
    return out, res


def kernel(**inputs):
    out, _ = run(inputs, trace=False)
    return out
